# revision 1
# baseline (speedup 1.0000x reference)
"""NT-Xent contrastive loss on 8 Trainium2 NeuronCores.

Strategy (row-sharded sim matrix, no collectives):
  - Every core receives the FULL proj_1/proj_2 plus its own 1024-row slice
    (rows_a) of concat(z_i, z_j) and the matching partner slice (rows_b).
  - Each core normalizes all 8192 rows, builds the transposed bf16
    embedding matrix zT [128d x 8192] as 16 [128,512] chunk tiles,
    computes its 1024x8192 block of exp(2*sim) row-sums via PE matmuls +
    ACT exp-with-accumulate, and emits per-row partials ln(denom) - 2*pos.
  - Host sums the 8 partial outputs -> scalar loss.

The ACT (scalar) engine carries the irreducible 8.4M exp evaluations per
core (~62us busy), so everything else stays off it: norms on DVE,
transposes on PE (first half, before the main loop owns PSUM) and on the
DMA xbar (second half, overlapped under the exp stream). 1/sqrt(n2) is
exp(-0.5*ln(n2)) computed in two batched Ln+Exp pairs -- Ln and Exp live
in different ACT table sets in this toolchain (a set switch costs ~1.3us),
so per-tile inv ops would thrash the table loader; batching caps it at
~5 switches, with data deps (not scheduler hints) enforcing the order.
"""

import sys

sys.path.insert(0, "/opt/trn_rl_repo")

import numpy as np

BATCH = 4096
DIM = 128
NCORES = 8
RPC = 2 * BATCH // NCORES  # 1024 rows per core
E2 = float(np.exp(2.0))  # exp(sim_gg / T) for the masked diagonal, sim_gg == 1

_CACHE = {}


def _build_nc():
    import concourse.bacc as bacc
    import concourse.bass as bass
    import concourse.mybir as mybir
    import concourse.tile as tile

    fp32 = mybir.dt.float32
    bf16 = mybir.dt.bfloat16
    AF = mybir.ActivationFunctionType
    ALU = mybir.AluOpType
    AX = mybir.AxisListType

    nc = bacc.Bacc("TRN2", target_bir_lowering=False, debug=False, num_devices=NCORES)
    p1 = nc.declare_dram_parameter("proj_1", [BATCH, DIM], fp32, isOutput=False)
    p2 = nc.declare_dram_parameter("proj_2", [BATCH, DIM], fp32, isOutput=False)
    ra = nc.declare_dram_parameter("rows_a", [RPC, DIM], fp32, isOutput=False)
    rb = nc.declare_dram_parameter("rows_b", [RPC, DIM], fp32, isOutput=False)
    out = nc.declare_dram_parameter("partial", [128, 8], fp32, isOutput=True)

    with tile.TileContext(nc) as tc:
        with (
            tc.tile_pool(name="big", bufs=1) as big,
            tc.tile_pool(name="jk", bufs=3) as jk,
        ):
            # DRAM views: contiguous per partition (partition p <- 8 rows/tile)
            src1 = p1[:].rearrange("(p a) d -> p (a d)", p=128)
            src2 = p2[:].rearrange("(p a) d -> p (a d)", p=128)
            srca = ra[:].rearrange("(p a) d -> p (a d)", p=128)
            srcb = rb[:].rearrange("(p a) d -> p (a d)", p=128)

            # source tiles [128, 1024]; load order: group1 first, xb last
            g1_names = ["xa", "x1_0", "x1_1", "x1_2", "x1_3"]
            g2_names = ["x2_0", "x2_1", "x2_2", "x2_3", "xb"]
            srcs = {"xa": srca, "xb": srcb}
            for m in range(4):
                srcs[f"x1_{m}"] = src1[:, 1024 * m : 1024 * (m + 1)]
                srcs[f"x2_{m}"] = src2[:, 1024 * m : 1024 * (m + 1)]
            xt = {}
            for name in g1_names + g2_names:
                t = big.tile([128, 1024], fp32, tag=name)
                nc.sync.dma_start(t[:], srcs[name])
                xt[name] = t

            # zT chunk tiles [128, 512] bf16 + zaT chunks
            zTc = []
            for c in range(16):
                zTc_t = big.tile([128, 512], bf16, tag=f"zT{c}")
                zTc.append(zTc_t)
            zaTc = []
            for c in range(2):
                zaTc_t = big.tile([128, 512], bf16, tag=f"zaT{c}")
                zaTc.append(zaTc_t)

            # identity for PE transposes (built on otherwise-idle GPSIMD)
            ident = big.tile([128, 128], bf16, tag="ident")
            ones1 = big.tile([128, 128], bf16, tag="ones1")
            nc.gpsimd.memset(ones1[:], 1.0)
            nc.gpsimd.affine_select(
                ident[:], ones1[:], [[1, 128]], ALU.is_equal, 0.0,
                base=0, channel_multiplier=-1,
            )

            def norms_into(n2g, names):
                for i, name in enumerate(names):
                    sq = jk.tile([128, 1024], fp32, tag="sq")
                    nc.gpsimd.tensor_mul(sq[:], xt[name][:], xt[name][:])
                    nc.vector.tensor_reduce(
                        n2g[:, 8 * i : 8 * (i + 1)],
                        sq[:].rearrange("p (a d) -> p a d", d=128),
                        axis=AX.X, op=ALU.add,
                    )

            def inv_of(n2g, label, w=40):
                lng = big.tile([128, w], fp32, tag=f"ln_{label}")
                nc.scalar.activation(lng[:], n2g[:], AF.Ln)
                invg = big.tile([128, w], fp32, tag=f"inv_{label}")
                nc.scalar.activation(invg[:], lng[:], AF.Exp, scale=-0.5)
                return invg

            def scale_tile(name, invg, i):
                z = big.tile([128, 1024], bf16, tag=f"z_{name}")
                for j in range(8):
                    nc.vector.tensor_scalar(
                        z[:, 128 * j : 128 * (j + 1)],
                        xt[name][:, 128 * j : 128 * (j + 1)],
                        invg[:, 8 * i + j : 8 * i + j + 1], None, op0=ALU.mult,
                    )
                return z

            def pe_transpose_group(z, b0, dst):
                pt = tp.tile([128, 512], fp32, tag="pt")
                for q in range(4):
                    nc.tensor.matmul(
                        pt[:, 128 * q : 128 * (q + 1)],
                        z[:, 128 * (b0 + q) : 128 * (b0 + q + 1)],
                        ident[:], start=True, stop=True,
                    )
                nc.vector.tensor_copy(dst[:], pt[:])

            # ---- group 1: xa + x1 -> inv -> scales -> PE transposes
            # (two inv batches so early zT chunks unblock the PE FIFO sooner)
            n2g1a = big.tile([128, 24], fp32, tag="n2g1a")
            norms_into(n2g1a, g1_names[:3])
            invg1 = inv_of(n2g1a, "g1a", 24)
            n2g1b = big.tile([128, 16], fp32, tag="n2g1b")
            norms_into(n2g1b, g1_names[3:])
            invg1b = inv_of(n2g1b, "g1b", 16)
            with tc.tile_pool(name="tp", bufs=3, space=bass.MemorySpace.PSUM) as tp:
                za = scale_tile("xa", invg1, 0)
                pe_transpose_group(za, 0, zaTc[0])
                pe_transpose_group(za, 4, zaTc[1])
                for m in range(2):
                    z = scale_tile(f"x1_{m}", invg1, m + 1)
                    pe_transpose_group(z, 0, zTc[2 * m])
                    pe_transpose_group(z, 4, zTc[2 * m + 1])
            # late x1 tiles go through the DMA xbar like x2 (keeps the PSUM
            # transpose pool short-lived so the main loop's pool starts early)
            for m in (2, 3):
                z = scale_tile(f"x1_{m}", invg1b, m - 2)
                for b in range(8):
                    c = 2 * m + b // 4
                    nc.sync.dma_start_transpose(
                        zTc[c][:, 128 * (b % 4) : 128 * (b % 4 + 1)],
                        z[:, 128 * b : 128 * (b + 1)],
                    )

            # ---- group 2: x2 + xb -> inv -> scales -> DMA-xbar transposes
            n2g2 = big.tile([128, 40], fp32, tag="n2g2")
            norms_into(n2g2, g2_names)
            invg2 = inv_of(n2g2, "g2")
            for m in range(4):
                z = scale_tile(f"x2_{m}", invg2, m)
                for b in range(8):
                    c = 8 + 2 * m + b // 4
                    nc.sync.dma_start_transpose(
                        zTc[c][:, 128 * (b % 4) : 128 * (b % 4 + 1)],
                        z[:, 128 * b : 128 * (b + 1)],
                    )

            # positives: D[p,j] = rows_a[8p+j] . rows_b[8p+j] (raw fp32 dots)
            pd = jk.tile([128, 1024], fp32, tag="sq")
            nc.vector.tensor_mul(pd[:], xt["xa"][:], xt["xb"][:])
            D = big.tile([128, 8], fp32, tag="D")
            nc.vector.tensor_reduce(
                D[:], pd[:].rearrange("p (a d) -> p a d", d=128),
                axis=AX.X, op=ALU.add,
            )

            # ---- main loop: phase A (h=0,1) uses zT chunks 0..7 (PE),
            # phase B (h=2,3) uses chunks 8..15 (DMA xbar, arriving meanwhile)
            RS = big.tile([128, 32], fp32, tag="RS")
            with tc.tile_pool(name="psum", bufs=2, space=bass.MemorySpace.PSUM) as pp:
                for h in (0, 1, 2, 3):
                    for j in range(8):
                        lhsT = zaTc[j // 4][:, 128 * (j % 4) : 128 * (j % 4 + 1)]
                        ps = pp.tile([128, 2048], fp32, tag="ps")
                        for q in range(4):
                            c = 4 * h + q
                            nc.tensor.matmul(
                                ps[:, 512 * q : 512 * (q + 1)], lhsT, zTc[c][:],
                                start=True, stop=True,
                            )
                        je = jk.tile([128, 2048], bf16, tag="je")
                        nc.scalar.activation(
                            je[:], ps[:], AF.Exp, scale=2.0,
                            accum_out=RS[:, 4 * j + h : 4 * j + h + 1],
                        )

            # ---- tail: partial[p,j] = ln(rowsum - e^2) - 2 * pos
            rs8 = big.tile([128, 8], fp32, tag="rs8")
            nc.vector.tensor_reduce(
                rs8[:], RS[:].rearrange("p (a c) -> p a c", c=4),
                axis=AX.X, op=ALU.add,
            )
            lnv = big.tile([128, 8], fp32, tag="lnv")
            nege2 = big.tile([128, 1], fp32, tag="nege2")
            nc.gpsimd.memset(nege2[:], -E2)
            nc.scalar.activation(lnv[:], rs8[:], AF.Ln, bias=nege2[:])
            t1 = big.tile([128, 8], fp32, tag="t1")
            nc.vector.tensor_mul(t1[:], D[:], invg1[:, 0:8])
            pos2 = big.tile([128, 8], fp32, tag="pos2")
            nc.vector.tensor_mul(pos2[:], t1[:], invg2[:, 32:40])
            p2t = big.tile([128, 8], fp32, tag="p2t")
            nc.vector.tensor_scalar(p2t[:], pos2[:], 2.0, None, op0=ALU.mult)
            res = big.tile([128, 8], fp32, tag="res")
            nc.vector.tensor_sub(res[:], lnv[:], p2t[:])
            nc.sync.dma_start(out[:], res[:])

    nc.compile()
    return nc


def _get_nc():
    if "nc" not in _CACHE:
        _CACHE["nc"] = _build_nc()
    return _CACHE["nc"]


def _in_maps(proj_1, proj_2):
    p1 = np.ascontiguousarray(np.asarray(proj_1, dtype=np.float32))
    p2 = np.ascontiguousarray(np.asarray(proj_2, dtype=np.float32))
    X = np.concatenate([p1, p2], axis=0)
    maps = []
    for k in range(NCORES):
        g0 = RPC * k
        pg = g0 + BATCH if g0 < BATCH else g0 - BATCH
        maps.append(
            {
                "proj_1": p1,
                "proj_2": p2,
                "rows_a": np.ascontiguousarray(X[g0 : g0 + RPC]),
                "rows_b": np.ascontiguousarray(X[pg : pg + RPC]),
            }
        )
    return maps


def _run(proj_1, proj_2, trace=False):
    from concourse.bass_utils import run_bass_kernel_spmd

    nc = _get_nc()
    res = run_bass_kernel_spmd(
        nc, _in_maps(proj_1, proj_2), list(range(NCORES)), trace=trace
    )
    tot = 0.0
    for k in range(NCORES):
        tot += float(res.results[k]["partial"].sum(dtype=np.float64))
    loss = np.float32(tot / (2 * BATCH))
    return loss, res


def kernel(proj_1, proj_2):
    loss, _ = _run(proj_1, proj_2, trace=False)
    return loss

